# revision 1
# baseline (speedup 1.0000x reference)
"""AttentionBlock (GroupNorm + MHSA + proj + residual) on 8 Trainium2 cores.

Sharding: pure data-parallel over batch (B=8, one batch element per core).
Per-core dataflow (C=512, N=H*W=1024, 8 heads, hd=64, 32 groups):
  1. GroupNorm stats via bn_stats per channel + PE group-mix matmul,
     affine folded with norm_w/norm_b -> xn (bf16).
  2. qkv: q,k via W^T matmul (q pre-scaled by hd^-0.5 on host);
     V^T computed directly as xn^T @ wv^T so the PV matmul needs no
     transposes on the critical path.
  3. Scores computed TRANSPOSED: S^T[m,n] = K^T Q (row-packed head
     pairs), exp on ACT (psum->sbuf bf16, 4-bank reads).
  4. O' = V^T-stationary matmul over P'^T (col-packed head pairs);
     softmax denominators ride as concurrent M=1 ones-matmuls into a
     shared psum bank (4-way col-tiled per quad).
  5. 1/s via DVE reciprocal_approx_accurate; partition-broadcast via a
     constant selection matmul; normalize during O' psum exit.
  6. proj matmul + bias + residual fused in one scalar_tensor_tensor.
"""

import numpy as np
import ml_dtypes

import concourse.bass as bass
import concourse.tile as tile
from concourse import mybir
from concourse.bass_utils import run_bass_kernel_spmd
from concourse.vector_clock import ScopedClock, VectorClock

f32 = mybir.dt.float32
bf16 = mybir.dt.bfloat16
BF16 = ml_dtypes.bfloat16

_RECIP_MODE = "lnexp"
_PSB_BUFS = 3
_PP_BUFS = 20
_RQ_BUFS = 2
_RB_ALIAS = True
_DENOM = True
B, C, N = 8, 512, 1024
NH, HD, G = 8, 64, 32
EPS = 1e-5
CC = C // 128  # 4 channel chunks
OC_QK = 8      # q+k output chunks (1024 rows)
NC = 2         # n in two 512-windows
MC = 8         # m in eight 128-chunks


def _split_multi_waits(bir_json):
    """This container's walrus build encodes at most one sync-wait command
    per TPB instruction. Engines execute in program order, so any extra
    waits can ride on NoOp instructions inserted immediately before the
    original instruction on the same engine (strictly more conservative
    ordering, semantics preserved)."""
    import orjson

    m = orjson.loads(bir_json)
    nop_id = [0]
    for fn in m.get("functions", []):
        for bb in fn.get("blocks", []):
            insts = bb.get("instructions", [])
            out = []
            for ins in insts:
                si = ins.get("sync_info") or {}
                waits = si.get("on_wait") or []
                eng = ins.get("engine", "Unassigned")
                if len(waits) > 1 and eng != "Unassigned":
                    for w in waits[:-1]:
                        nop_id[0] += 1
                        out.append(
                            {
                                "debug": ins.get("debug", 0),
                                "engine": eng,
                                "ins": [],
                                "outs": [],
                                "name": f"{ins['name']}-w{nop_id[0]}",
                                "opcode": "NoOp",
                                "sync_info": {"on_wait": [w]},
                            }
                        )
                    si = dict(si)
                    si["on_wait"] = [waits[-1]]
                    ins = dict(ins)
                    ins["sync_info"] = si
                out.append(ins)
            bb["instructions"] = out
    return orjson.dumps(m)


def _patch_tile():
    """This container's walrus accepts few sem-waits per instruction; split
    TileContext's kernel-tail drain into one drain per pending proc and
    hoist any remaining multi-waits onto NoOps at compile time."""
    if getattr(tile.TileContext, "_drain_split_patched", False):
        return

    from concourse import bass2jax, bass_utils

    orig_compile = bass_utils.compile_bir_kernel

    def compile_with_split(bir_json, tmpdir, neff_name="file.neff"):
        return orig_compile(_split_multi_waits(bir_json), tmpdir, neff_name=neff_name)

    bass_utils.compile_bir_kernel = compile_with_split
    bass2jax.compile_bir_kernel = compile_with_split

    def _drain_and_barrier_split(self, tick_clock, wait_clock):
        gc = tick_clock.global_clock
        ticks = list(gc)
        for p, t in enumerate(ticks):
            if t <= 0:
                continue
            vec = [0] * len(ticks)
            vec[p] = t
            drain_inst = self.nc.sync.drain()
            wait_clock.add_sem_waits(
                drain_inst.ins, ScopedClock({None: VectorClock(vec)})
            )
        self.nc.all_engine_barrier()
        assert self.sems is not None
        popped = self.nc._tile_sem_poison_stack.pop()
        assert popped is self._sem_poison
        self.nc.clear_and_free_semaphores(list(self.sems.allocated().values()))
        self.nc.all_engine_barrier()

    tile.TileContext._drain_and_barrier = _drain_and_barrier_split
    tile.TileContext._drain_split_patched = True


def host_prep(x, norm_w, norm_b, qkv_w, qkv_b, proj_w, proj_b):
    """Host-side layout/dtype prep. Pure layout transforms + folding the
    1/sqrt(hd) attention scale into W_q/b_q (exact: 0.125 is a power of 2)."""
    x = np.ascontiguousarray(np.asarray(x, np.float32)).reshape(B, C, N)
    qkv_w = np.asarray(qkv_w, np.float32)
    qkv_b = np.asarray(qkv_b, np.float32)
    scale = float(HD) ** -0.5

    wqk = qkv_w[: 2 * C].copy()
    wqk[:C] *= scale
    bqk = qkv_b[: 2 * C].copy()
    bqk[:C] *= scale

    common = {
        "wqkT": np.ascontiguousarray(wqk.T).astype(BF16),            # [512,1024]
        "wvT": np.ascontiguousarray(qkv_w[2 * C :].T).astype(BF16),  # [512,512]
        "wpT": np.ascontiguousarray(np.asarray(proj_w, np.float32).T).astype(BF16),
        "bqk": np.ascontiguousarray(bqk.reshape(OC_QK, 128).T).astype(np.float32),
        "bv_row": qkv_b[2 * C :].reshape(1, C).astype(BF16),
        "bp": np.ascontiguousarray(
            np.asarray(proj_b, np.float32).reshape(CC, 128).T
        ).astype(np.float32),
        "nw": np.ascontiguousarray(
            np.asarray(norm_w, np.float32).reshape(CC, 128).T
        ).astype(np.float32),
        "nb": np.ascontiguousarray(
            np.asarray(norm_b, np.float32).reshape(CC, 128).T
        ).astype(np.float32),
        "gmat": _gmat(),
        "pselA": _psel(0, 32),
        "pselB": _psel(64, 96),
        "ones_col": np.ones((128, 1), BF16),
        "ones_row": np.ones((1, 128), BF16),
    }
    return common, [x[i] for i in range(B)]


def _gmat():
    g = np.zeros((128, 128), np.float32)
    per = 128 // 16  # channels per group = 16 -> 8 groups per 128-chunk
    for i in range(128):
        gi = i // 16
        g[i, gi * 16 : (gi + 1) * 16] = 1.0 / (16.0 * 1024.0)
    return g


def _psel(r0, r1):
    p = np.zeros((128, 128), np.float32)
    p[r0, 0:64] = 1.0
    p[r1, 64:128] = 1.0
    return p


def build_nc(unroll=1):
    _patch_tile()
    nc = bass.Bass()
    d = {}
    d["x"] = nc.declare_dram_parameter("x", [C, N], f32, isOutput=False)
    d["wqkT"] = nc.declare_dram_parameter("wqkT", [C, 2 * C], bf16, isOutput=False)
    d["wvT"] = nc.declare_dram_parameter("wvT", [C, C], bf16, isOutput=False)
    d["wpT"] = nc.declare_dram_parameter("wpT", [C, C], bf16, isOutput=False)
    d["bqk"] = nc.declare_dram_parameter("bqk", [128, OC_QK], f32, isOutput=False)
    d["bv_row"] = nc.declare_dram_parameter("bv_row", [1, C], bf16, isOutput=False)
    d["bp"] = nc.declare_dram_parameter("bp", [128, CC], f32, isOutput=False)
    d["nw"] = nc.declare_dram_parameter("nw", [128, CC], f32, isOutput=False)
    d["nb"] = nc.declare_dram_parameter("nb", [128, CC], f32, isOutput=False)
    d["gmat"] = nc.declare_dram_parameter("gmat", [128, 128], f32, isOutput=False)
    d["pselA"] = nc.declare_dram_parameter("pselA", [128, 128], f32, isOutput=False)
    d["pselB"] = nc.declare_dram_parameter("pselB", [128, 128], f32, isOutput=False)
    d["ones_col"] = nc.declare_dram_parameter("ones_col", [128, 1], bf16, isOutput=False)
    d["ones_row"] = nc.declare_dram_parameter("ones_row", [1, 128], bf16, isOutput=False)
    d["out"] = nc.declare_dram_parameter("out", [C, N], f32, isOutput=True)

    with tile.TileContext(nc) as tc:
        with (
            tc.tile_pool(name="sing", bufs=1) as sing,
            tc.tile_pool(name="gn", bufs=4) as gnp,
            tc.tile_pool(name="pp", bufs=_PP_BUFS) as ppp,
            tc.tile_pool(name="rqp", bufs=_RQ_BUFS) as rqp,
            tc.tile_pool(name="psA", bufs=2, space="PSUM") as psA,
            tc.tile_pool(name="psB", bufs=_PSB_BUFS, space="PSUM") as psB,
            tc.tile_pool(name="psD", bufs=1, space="PSUM") as psD,
        ):
            # ---- constants (loaded once, shared by all unrolled iters) ----
            cst = {}
            for name, shape, dt in (
                ("wqkT", [128, CC, 2 * C], bf16),
                ("wvT", [128, CC, C], bf16),
                ("wpT", [128, CC, C], bf16),
                ("bqk", [128, OC_QK], f32),
                ("bv_row", [1, C], bf16),
                ("bp", [128, CC], f32),
                ("nw", [128, CC], f32),
                ("nb", [128, CC], f32),
                ("gmat", [128, 128], f32),
                ("pselA", [128, 128], f32),
                ("pselB", [128, 128], f32),
                ("ones_col", [128, 1], bf16),
                ("ones_row", [1, 128], bf16),
            ):
                t = sing.tile(shape, dt, tag=name, name=name)
                src = d[name]
                if len(shape) == 3:
                    # chunked weights: [c, o] dram -> [128, cc, o] sbuf
                    nc.sync.dma_start(
                        out=t, in_=src.rearrange("(cc p) o -> p cc o", p=128)
                    )
                else:
                    nc.sync.dma_start(out=t, in_=src[:])
                cst[name] = t
            eps_t = sing.tile([128, 1], f32, tag="eps", name="eps")
            nc.vector.memset(eps_t, EPS)

            s_ps = psD.tile([128, 512], f32, tag="s", name="s_ps")
            if _RB_ALIAS:
                # rb broadcasts reuse the sums bank after the reciprocal reads it
                rb_ps = s_ps
            else:
                rb_ps = psD.tile([128, 512], f32, tag="rb", name="rb_ps")
                nc.vector.memset(rb_ps, 1.0)
            # stale psum rows multiply by zero weights in the psel matmul and
            # pass through reciprocal; they must be finite, never NaN.
            nc.vector.memset(s_ps, 1.0)

            for _ in range(unroll):
                _body(nc, tc, d, cst, sing, gnp, ppp, rqp, psA, psB, s_ps, rb_ps, eps_t)
    return nc


def _body_staged(nc, tc, d, cst, sing, gnp, ppp, rqp, psA, psB, s_ps, rb_ps, eps_t, stage):
    _body(nc, tc, d, cst, sing, gnp, ppp, rqp, psA, psB, s_ps, rb_ps, eps_t, stage=stage)


def _body(nc, tc, d, cst, sing, gnp, ppp, rqp, psA, psB, s_ps, rb_ps, eps_t, stage="full"):
    AF = mybir.ActivationFunctionType
    global AF_LN, AF_EXP
    AF_LN, AF_EXP = AF.Ln, AF.Exp
    OP = mybir.AluOpType

    x4 = []
    for cc in range(CC):
        xt = sing.tile([128, N], f32, tag=f"x{cc}", name=f"x{cc}")
        nc.gpsimd.dma_start(out=xt, in_=d["x"][cc * 128 : (cc + 1) * 128, :])
        x4.append(xt)

    # ---------------- GroupNorm -> xn (bf16) ----------------
    xn = []
    sq = gnp.tile([128, N], f32, tag="sq", name="sq", bufs=2)
    for cc in range(CC):
        # ms = [sum(x), sum(x^2)] per channel; gmat folds the /16384
        ms = gnp.tile([128, 2], f32, tag="ms", name="ms")
        nc.vector.tensor_reduce(
            out=ms[:, 0:1], in_=x4[cc], axis=mybir.AxisListType.X, op=OP.add
        )
        nc.scalar.activation(
            out=sq, in_=x4[cc], func=AF.Square, accum_out=ms[:, 1:2]
        )
        gst_ps = psB.tile([128, 2], f32, tag="bank", name="gst_ps")
        nc.tensor.matmul(gst_ps, cst["gmat"], ms, start=True, stop=True)
        gst = gnp.tile([128, 2], f32, tag="gst", name="gst")
        nc.vector.tensor_copy(out=gst, in_=gst_ps)
        # negvar = mean_g^2 - E_g[x^2]  (rstd uses scale=-1 to flip sign)
        negvar = gnp.tile([128, 1], f32, tag="negvar", name="negvar")
        nc.vector.scalar_tensor_tensor(
            out=negvar,
            in0=gst[:, 0:1],
            scalar=gst[:, 0:1],
            in1=gst[:, 1:2],
            op0=OP.mult,
            op1=OP.subtract,
        )
        rstd = gnp.tile([128, 1], f32, tag="rstd", name="rstd")
        nc.scalar.activation(out=rstd, in_=negvar, func=AF.Sqrt, bias=eps_t, scale=-1.0)
        nc.vector.reciprocal(out=rstd, in_=rstd)
        aa = gnp.tile([128, 1], f32, tag="aa", name="aa")
        nc.vector.tensor_mul(out=aa, in0=rstd, in1=cst["nw"][:, cc : cc + 1])
        # bbn = mean_g*A - norm_b   (applied as x*A - bbn)
        bbn = gnp.tile([128, 1], f32, tag="bbn", name="bbn")
        nc.vector.scalar_tensor_tensor(
            out=bbn,
            in0=gst[:, 0:1],
            scalar=aa,
            in1=cst["nb"][:, cc : cc + 1],
            op0=OP.mult,
            op1=OP.subtract,
        )
        xnt = sing.tile([128, N], bf16, tag=f"xn{cc}", name=f"xn{cc}")
        nc.vector.tensor_scalar(
            out=xnt, in0=x4[cc], scalar1=aa, scalar2=bbn, op0=OP.mult, op1=OP.subtract
        )
        xn.append(xnt)

    if stage == "gn":
        return

    # ---------------- q, k generation ----------------
    qk = []
    for oc in range(OC_QK):
        qkt = sing.tile([128, N], bf16, tag=f"qk{oc}", name=f"qk{oc}")
        qk.append(qkt)
        for nci in range(NC):
            ps = psB.tile([128, 512], f32, tag="bank", name="bank")
            for cc in range(CC):
                nc.tensor.matmul(
                    ps,
                    cst["wqkT"][:, cc, oc * 128 : (oc + 1) * 128],
                    xn[cc][:, nci * 512 : (nci + 1) * 512],
                    start=(cc == 0),
                    stop=(cc == CC - 1),
                )
            nc.vector.tensor_scalar_add(
                out=qkt[:, nci * 512 : (nci + 1) * 512],
                in0=ps,
                scalar1=cst["bqk"][:, oc : oc + 1],
            )

    if stage == "qk":
        return

    # ---------------- V^T generation: V^T[n, vo] = xn^T @ wv^T ----------------
    vT = []
    for mci in range(MC):
        vt = sing.tile([128, C], bf16, tag=f"vT{mci}", name=f"vT{mci}")
        vT.append(vt)
        ps = psB.tile([128, 512], f32, tag="bank", name="bank")
        for cc in range(CC):
            nc.tensor.matmul(
                ps,
                xn[cc][:, mci * 128 : (mci + 1) * 128],
                cst["wvT"][:, cc, :],
                start=(cc == 0),
                stop=False,
            )
        nc.tensor.matmul(ps, cst["ones_row"], cst["bv_row"], start=False, stop=True)
        nc.vector.tensor_copy(out=vt, in_=ps)

    if stage == "vt":
        return

    # ---------------- attention ----------------
    o4 = [sing.tile([128, N], bf16, tag=f"o{cc}", name=f"o{cc}") for cc in range(CC)]

    for nci in range(NC):
        nwin = slice(nci * 512, (nci + 1) * 512)
        for q in range(2):  # quad of heads 4q..4q+3
            pp_tiles = {}
            for pi in range(2):  # pair within quad
                h0 = 4 * q + 2 * pi       # even head -> partitions 0:64
                h1 = h0 + 1               # odd head  -> partitions 64:128
                for mc in range(MC):
                    sg = psA.tile([128, 1024], f32, tag="sg", name="sg")
                    for sl, h in enumerate((h0, h1)):
                        hp = (h % 2) * 64
                        nc.tensor.matmul(
                            sg[:, sl * 512 : (sl + 1) * 512],
                            qk[4 + h // 2][hp : hp + 64, mc * 128 : (mc + 1) * 128],
                            qk[h // 2][hp : hp + 64, nwin],
                            start=True,
                            stop=True,
                            tile_position=(hp, 0),
                        )
                    pt = ppp.tile([128, 1024], bf16, tag="pp", name="pp")
                    nc.scalar.activation(out=pt, in_=sg, func=AF.Exp)
                    pp_tiles[(pi, mc)] = pt

            if stage == "scores":
                continue

            att = [psB.tile([128, 512], f32, tag="bank", name="bank") for _ in range(2)]
            for mc in range(MC):
                for pi in range(2):
                    h0 = 4 * q + 2 * pi
                    pt = pp_tiles[(pi, mc)]
                    for hh in range(2):  # head within pair
                        nc.tensor.matmul(
                            att[pi][hh * 64 : (hh + 1) * 64, :],
                            vT[mc][:, (h0 + hh) * 64 : (h0 + hh + 1) * 64],
                            pt[:, hh * 512 : (hh + 1) * 512],
                            start=(mc == 0),
                            stop=(mc == MC - 1),
                            tile_position=(0, hh * 64),
                            skip_group_check=True,
                        )
            # denominators after: 4-way col-tiled M=1 ones matmuls, so the
            # attnV stream is not blocked by the s_ps recip/rbc chain
            for mc in range(MC if _DENOM else 0):
                for pi in range(2):
                    pt = pp_tiles[(pi, mc)]
                    for hh in range(2):
                        j = 2 * pi + hh
                        nc.tensor.matmul(
                            s_ps[32 * j : 32 * j + 1, :],
                            cst["ones_col"],
                            pt[:, hh * 512 : (hh + 1) * 512],
                            start=(mc == 0),
                            stop=(mc == MC - 1),
                            tile_position=(0, 32 * j),
                            skip_group_check=True,
                        )

            if stage == "att" or not _DENOM:
                for pi in range(2):
                    nc.vector.tensor_copy(out=o4[2 * q + pi][:, nwin], in_=att[pi])
                continue

            rq = rqp.tile([128, 512], f32, tag="rq", name="rq")
            scr = rqp.tile([128, 512], f32, tag="scr", name="scr")
            if _RECIP_MODE == "lnexp":
                # r = exp(-ln(s)) on ACT: both functions live in the
                # natural_log_exp_and_others table set
                nc.scalar.activation(out=scr, in_=s_ps, func=AF_LN)
                nc.scalar.activation(out=rq, in_=scr, func=AF_EXP, scale=-1.0)
            elif _RECIP_MODE == "approx":
                nc.vector.reciprocal_approx_accurate(out=rq, in_=s_ps, scratch=scr)
            else:
                nc.vector.reciprocal(out=rq, in_=s_ps)

            for pi in range(2):
                # O' exit (unnormalized) straight into the O channel tiles
                nc.vector.tensor_copy(out=o4[2 * q + pi][:, nwin], in_=att[pi])
                # broadcast r rows to 64-partition blocks, then normalize
                psel = cst["pselA"] if pi == 0 else cst["pselB"]
                nc.tensor.matmul(rb_ps, psel, rq, start=True, stop=True)
                nc.vector.tensor_tensor(
                    out=o4[2 * q + pi][:, nwin],
                    in0=o4[2 * q + pi][:, nwin],
                    in1=rb_ps,
                    op=mybir.AluOpType.mult,
                )
            if stage == "recip":
                continue

    if stage in ("scores", "att", "recip"):
        return

    # ---------------- proj + bias + residual (per n-window) ----------------
    for nci in range(NC):
        nwin = slice(nci * 512, (nci + 1) * 512)
        for oc in range(CC):
            ps = psB.tile([128, 512], f32, tag="bank", name="bank")
            for cc in range(CC):
                nc.tensor.matmul(
                    ps,
                    cst["wpT"][:, cc, oc * 128 : (oc + 1) * 128],
                    o4[cc][:, nwin],
                    start=(cc == 0),
                    stop=(cc == CC - 1),
                )
            ob = gnp.tile([128, 512], f32, tag="ob", name="ob", bufs=4)
            nc.vector.scalar_tensor_tensor(
                out=ob,
                in0=ps,
                scalar=cst["bp"][:, oc : oc + 1],
                in1=x4[oc][:, nwin],
                op0=OP.add,
                op1=OP.add,
            )
            nc.sync.dma_start(out=d["out"][oc * 128 : (oc + 1) * 128, nwin], in_=ob)


_BUILT = None


def kernel(**inputs):
    global _BUILT
    common, xs = host_prep(**inputs)
    if _BUILT is None:
        _BUILT = build_nc(unroll=1)
    nc = _BUILT
    in_maps = [dict(common, x=xs[i]) for i in range(B)]
    res = run_bass_kernel_spmd(nc, in_maps, core_ids=list(range(B)))
    out = np.stack([res.results[i]["out"] for i in range(B)], axis=0)
    return out.reshape(B, C, 32, 32).astype(np.float32)



# revision 9
# speedup vs baseline: 1.1708x; 1.1708x over previous
"""AttentionBlock (GroupNorm + MHSA + proj + residual) on 8 Trainium2 cores.

Sharding: pure data-parallel over batch (B=8, one batch element per core).

v4 — ACT-exp drumbeat + fp8-DoubleRow attention:
  * The softmax exp stream (64 x [128,1024] activations = 8.4M elems/core)
    is the hard floor (~66us on ACT at 1 elem/cycle/lane).  Everything
    else hides under it.
  * P (exp output) is stored e5m2 (covers exp([-7.3, 7.3]) with huge
    range margin); V^T / proj weights / o4 are e4m3; q/k stay bf16
    (fp8 q/k costs ~1.2e-2 alone via exp amplification).
  * attnV runs as full-width fp8 DoubleRow matmuls with HALF-ZERO
    stationaries [V_even | 0] / [0 | V_odd], which stacks both heads of
    a pair into one PSUM bank without tile_position (walrus rejects
    DoubleRow + tile_position).  Softmax denominators use the same
    trick: [0..0 | ones] stationaries of width 32j+1 land each pair's
    sum on partition row 32j of a shared pre-zeroed bank.
  * GroupNorm stats via DVE bn_stats/bn_aggr; softmax reciprocal on DVE
    (bf16 out) feeding bf16 psel broadcast matmuls.
  * Emission is a beat-woven pipeline: scores (2-way row-packed, bf16)
    pace the ACT exps; attnV/denominator DR matmuls trail two beats
    behind; one "filler" unit per beat carries the previous block's
    recip/psel/normalize tail, proj of the previous window, and the
    next iteration's GN/qkv/V^T, with parity-double-buffered tiles.
"""

from collections import deque

import numpy as np
import ml_dtypes

import concourse.bass as bass
import concourse.tile as tile
from concourse import mybir
from concourse.bass_utils import run_bass_kernel_spmd
from concourse.vector_clock import ScopedClock, VectorClock

f32 = mybir.dt.float32
bf16 = mybir.dt.bfloat16
f8 = mybir.dt.float8e4
f8e5 = mybir.dt.float8e5
BF16 = ml_dtypes.bfloat16
F8NP = mybir.dt.np(f8)
DR = mybir.MatmulPerfMode.DoubleRow

B, C, N = 8, 512, 1024
NH, HD, G = 8, 64, 32
EPS = 1e-5
CC = C // 128   # 4 channel chunks
KK = CC // 2    # 2 DoubleRow chunk-pairs
OC_QK = 8       # q+k output chunks (1024 rows)
NC = 2          # n in two 512-windows
MC = 8          # m in eight 128-chunks
MCP = MC // 2   # 4 DoubleRow m-chunk-pairs
F8MAX = 240.0
S_EPS = 1e-30   # keeps unused s_ps rows finite through the reciprocal


def _split_multi_waits(bir_json):
    """This container's walrus build encodes at most one sync-wait command
    per TPB instruction. Engines execute in program order, so any extra
    waits can ride on NoOp instructions inserted immediately before the
    original instruction on the same engine (strictly more conservative
    ordering, semantics preserved)."""
    import orjson

    m = orjson.loads(bir_json)
    nop_id = [0]
    for fn in m.get("functions", []):
        for bb in fn.get("blocks", []):
            insts = bb.get("instructions", [])
            out = []
            for ins in insts:
                si = ins.get("sync_info") or {}
                waits = si.get("on_wait") or []
                eng = ins.get("engine", "Unassigned")
                if len(waits) > 1 and eng != "Unassigned":
                    for w in waits[:-1]:
                        nop_id[0] += 1
                        out.append(
                            {
                                "debug": ins.get("debug", 0),
                                "engine": eng,
                                "ins": [],
                                "outs": [],
                                "name": f"{ins['name']}-w{nop_id[0]}",
                                "opcode": "NoOp",
                                "sync_info": {"on_wait": [w]},
                            }
                        )
                    si = dict(si)
                    si["on_wait"] = [waits[-1]]
                    ins = dict(ins)
                    ins["sync_info"] = si
                out.append(ins)
            bb["instructions"] = out
    return orjson.dumps(m)


def _patch_tile():
    """This container's walrus accepts few sem-waits per instruction; split
    TileContext's kernel-tail drain into one drain per pending proc and
    hoist any remaining multi-waits onto NoOps at compile time."""
    if getattr(tile.TileContext, "_drain_split_patched", False):
        return

    from concourse import bass2jax, bass_utils

    orig_compile = bass_utils.compile_bir_kernel

    def compile_with_split(bir_json, tmpdir, neff_name="file.neff"):
        return orig_compile(_split_multi_waits(bir_json), tmpdir, neff_name=neff_name)

    bass_utils.compile_bir_kernel = compile_with_split
    bass2jax.compile_bir_kernel = compile_with_split

    def _drain_and_barrier_split(self, tick_clock, wait_clock):
        gc = tick_clock.global_clock
        ticks = list(gc)
        for p, t in enumerate(ticks):
            if t <= 0:
                continue
            vec = [0] * len(ticks)
            vec[p] = t
            drain_inst = self.nc.sync.drain()
            wait_clock.add_sem_waits(
                drain_inst.ins, ScopedClock({None: VectorClock(vec)})
            )
        self.nc.all_engine_barrier()
        assert self.sems is not None
        popped = self.nc._tile_sem_poison_stack.pop()
        assert popped is self._sem_poison
        self.nc.clear_and_free_semaphores(list(self.sems.allocated().values()))
        self.nc.all_engine_barrier()

    tile.TileContext._drain_and_barrier = _drain_and_barrier_split
    tile.TileContext._drain_split_patched = True


def _to_f8(a):
    return np.clip(np.asarray(a, np.float32), -F8MAX, F8MAX).astype(F8NP)


# even heads first, then odd heads: V^T psum columns come out pre-grouped
# for the [V_even | 0 | 0 | V_odd] stationary layout
_VPERM = [0, 2, 4, 6, 1, 3, 5, 7]


def host_prep(x, norm_w, norm_b, qkv_w, qkv_b, proj_w, proj_b):
    """Host-side layout/dtype prep.  Folds the 1/sqrt(hd) attention scale
    into W_q/b_q (exact: 0.125 is a power of 2) and the V bias through the
    softmax (rows sum to 1) into the projection bias:
    bp' = proj_b + proj_w @ b_v."""
    x = np.ascontiguousarray(np.asarray(x, np.float32)).reshape(B, C, N)
    qkv_w = np.asarray(qkv_w, np.float32)
    qkv_b = np.asarray(qkv_b, np.float32)
    proj_w = np.asarray(proj_w, np.float32)
    proj_b = np.asarray(proj_b, np.float32)
    scale = float(HD) ** -0.5

    wqk = qkv_w[: 2 * C].copy()
    wqk[:C] *= scale
    bqk = qkv_b[: 2 * C].copy()
    bqk[:C] *= scale
    bp_eff = proj_b + proj_w @ qkv_b[2 * C :]

    # permute V output channels: even heads' 64-blocks first, then odd
    wv = qkv_w[2 * C :].reshape(NH, HD, C)[_VPERM].reshape(C, C)

    common = {
        "wqkT": np.ascontiguousarray(wqk.T).astype(BF16),   # [512, 1024]
        "wvT": _to_f8(wv.T),                                # [512, 512]
        "wpT": _to_f8(proj_w.T),                            # [512, 512]
        "bqk": np.ascontiguousarray(bqk.reshape(OC_QK, 128).T).astype(np.float32),
        "bp": np.ascontiguousarray(bp_eff.reshape(CC, 128).T).astype(np.float32),
        "nw": np.ascontiguousarray(
            np.asarray(norm_w, np.float32).reshape(CC, 128).T
        ).astype(np.float32),
        "nb": np.ascontiguousarray(
            np.asarray(norm_b, np.float32).reshape(CC, 128).T
        ).astype(np.float32),
        "gmat": _gmat(),
        "pselA": _psel(0, 32).astype(BF16),
        "pselB": _psel(64, 96).astype(BF16),
    }
    return common, [x[i] for i in range(B)]


def _gmat():
    # mixes per-channel [mean, E[x^2]] into per-group averages (16 ch/group)
    g = np.zeros((128, 128), np.float32)
    for i in range(128):
        gi = i // 16
        g[i, gi * 16 : (gi + 1) * 16] = 1.0 / 16.0
    return g


def _psel(r0, r1):
    p = np.zeros((128, 128), np.float32)
    p[r0, 0:64] = 1.0
    p[r1, 64:128] = 1.0
    return p


def build_nc(unroll=1):
    _patch_tile()
    nc = bass.Bass()
    d = {}
    d["x"] = nc.declare_dram_parameter("x", [C, N], f32, isOutput=False)
    d["wqkT"] = nc.declare_dram_parameter("wqkT", [C, 2 * C], bf16, isOutput=False)
    d["wvT"] = nc.declare_dram_parameter("wvT", [C, C], f8, isOutput=False)
    d["wpT"] = nc.declare_dram_parameter("wpT", [C, C], f8, isOutput=False)
    d["bqk"] = nc.declare_dram_parameter("bqk", [128, OC_QK], f32, isOutput=False)
    d["bp"] = nc.declare_dram_parameter("bp", [128, CC], f32, isOutput=False)
    d["nw"] = nc.declare_dram_parameter("nw", [128, CC], f32, isOutput=False)
    d["nb"] = nc.declare_dram_parameter("nb", [128, CC], f32, isOutput=False)
    d["gmat"] = nc.declare_dram_parameter("gmat", [128, 128], f32, isOutput=False)
    d["pselA"] = nc.declare_dram_parameter("pselA", [128, 128], bf16, isOutput=False)
    d["pselB"] = nc.declare_dram_parameter("pselB", [128, 128], bf16, isOutput=False)
    d["out"] = nc.declare_dram_parameter("out", [C, N], f32, isOutput=True)

    OP = mybir.AluOpType
    AF = mybir.ActivationFunctionType

    with tile.TileContext(nc) as tc:
        with (
            tc.tile_pool(name="sing", bufs=1) as sing,
            tc.tile_pool(name="gn", bufs=4) as gnp,
            tc.tile_pool(name="pp", bufs=10) as ppp,
            tc.tile_pool(name="rqp", bufs=2) as rqp,
            tc.tile_pool(name="obp", bufs=4) as obp,
            tc.tile_pool(name="psA", bufs=2, space="PSUM") as psA,
            tc.tile_pool(name="psT", bufs=2, space="PSUM") as psT,
            tc.tile_pool(name="psW", bufs=1, space="PSUM") as psW,
            tc.tile_pool(name="psD", bufs=1, space="PSUM") as psD,
        ):
            # ---- constants ----
            cst = {}
            for name, shape, dt in (
                ("wqkT", [128, CC, 2 * C], bf16),
                ("wvT", [128, CC, C], f8),
                ("wpT", [128, CC, C], f8),
                ("bqk", [128, OC_QK], f32),
                ("bp", [128, CC], f32),
                ("nw", [128, CC], f32),
                ("nb", [128, CC], f32),
                ("gmat", [128, 128], f32),
                ("pselA", [128, 128], bf16),
                ("pselB", [128, 128], bf16),
            ):
                t = sing.tile(shape, dt, tag=name, name=name)
                src = d[name]
                if len(shape) == 3:
                    nc.sync.dma_start(
                        out=t, in_=src.rearrange("(cc p) o -> p cc o", p=128)
                    )
                else:
                    nc.sync.dma_start(out=t, in_=src[:])
                cst[name] = t
            eps_t = sing.tile([128, 1], f32, tag="eps", name="eps")
            nc.vector.memset(eps_t, EPS)
            # denominator stationaries: [0]*32j zeros then a ones column;
            # slice [:, :, 96-32j:97] is the M=32j+1 stationary for row 32j
            dones = sing.tile([128, 2, 112], f8, tag="dones", name="dones")
            nc.vector.memset(dones, 0.0)
            nc.vector.memset(dones[:, :, 111:112], 1.0)

            s_ps = psD.tile([128, 512], f32, tag="s", name="s_ps")
            rb_ps = s_ps  # rb broadcasts reuse the sums bank after recip
            nc.vector.memset(s_ps, S_EPS)

            # ---- per-parity state tiles ----
            x4, xnb, xn8, qk, vt3, o4 = {}, {}, {}, {}, {}, {}
            for p_ in range(2):
                x4[p_] = [
                    sing.tile([128, NC, 512], f32, tag=f"x{cc}_{p_}", name=f"x{cc}_{p_}")
                    for cc in range(CC)
                ]
                xnb[p_] = sing.tile([128, CC, N], bf16, tag=f"xnb_{p_}", name=f"xnb_{p_}")
                xn8[p_] = sing.tile([128, CC, N], f8, tag=f"xn8_{p_}", name=f"xn8_{p_}")
                qk[p_] = [
                    sing.tile([128, N], bf16, tag=f"qk{oc}_{p_}", name=f"qk{oc}_{p_}")
                    for oc in range(OC_QK)
                ]
                # [V_even | 0 | 0 | V_odd] per head-pair group of 256 cols
                vt3[p_] = sing.tile(
                    [128, MC, 4, 256], f8, tag=f"vt3_{p_}", name=f"vt3_{p_}"
                )
                nc.vector.memset(vt3[p_], 0.0)
                o4[p_] = sing.tile([128, CC, N], f8, tag=f"o4_{p_}", name=f"o4_{p_}")

            fillers = deque()

            def pop_filler():
                if fillers:
                    fillers.popleft()()

            # ---------------- stage emitters ----------------
            def gn_units(i):
                """GroupNorm of iteration i as a list of filler units."""
                p_ = i % 2
                units = []

                def load(cc):
                    def _u():
                        nc.gpsimd.dma_start(
                            out=x4[p_][cc],
                            in_=d["x"][cc * 128 : (cc + 1) * 128, :].rearrange(
                                "p (s n) -> p s n", s=NC
                            ),
                        )
                    return _u

                def stats(cc):
                    def _u():
                        xt = x4[p_][cc]
                        st6 = gnp.tile([128, NC * 6], f32, tag="st6", name="st6")
                        for h in range(NC):
                            nc.vector.bn_stats(st6[:, h * 6 : (h + 1) * 6], xt[:, h, :])
                        ms = gnp.tile([128, 2], f32, tag="ms", name="ms")
                        nc.vector.bn_aggr(ms, st6)
                        # ms = [mean_c, var_c] -> ex2 = [mean_c, E[x^2]_c]
                        ex2 = gnp.tile([128, 2], f32, tag="ex2", name="ex2")
                        nc.vector.tensor_copy(out=ex2[:, 0:1], in_=ms[:, 0:1])
                        nc.vector.scalar_tensor_tensor(
                            out=ex2[:, 1:2],
                            in0=ms[:, 0:1],
                            scalar=ms[:, 0:1],
                            in1=ms[:, 1:2],
                            op0=OP.mult,
                            op1=OP.add,
                        )
                        gst_ps = psW.tile([128, 2], f32, tag="w", name="gst_ps")
                        nc.tensor.matmul(gst_ps, cst["gmat"], ex2, start=True, stop=True)
                        gst = gnp.tile([128, 2], f32, tag="gst", name="gst")
                        nc.vector.tensor_copy(out=gst, in_=gst_ps)
                        negvar = gnp.tile([128, 1], f32, tag="negvar", name="negvar")
                        nc.vector.scalar_tensor_tensor(
                            out=negvar,
                            in0=gst[:, 0:1],
                            scalar=gst[:, 0:1],
                            in1=gst[:, 1:2],
                            op0=OP.mult,
                            op1=OP.subtract,
                        )
                        rstd = gnp.tile([128, 1], f32, tag="rstd", name="rstd")
                        nc.scalar.activation(
                            out=rstd, in_=negvar, func=AF.Sqrt, bias=eps_t, scale=-1.0
                        )
                        nc.vector.reciprocal(out=rstd, in_=rstd)
                        aa = gnp.tile([128, 1], f32, tag="aa", name="aa")
                        nc.vector.tensor_mul(out=aa, in0=rstd, in1=cst["nw"][:, cc : cc + 1])
                        bbn = gnp.tile([128, 1], f32, tag="bbn", name="bbn")
                        nc.vector.scalar_tensor_tensor(
                            out=bbn,
                            in0=gst[:, 0:1],
                            scalar=aa,
                            in1=cst["nb"][:, cc : cc + 1],
                            op0=OP.mult,
                            op1=OP.subtract,
                        )
                        for h in range(NC):
                            nc.vector.tensor_scalar(
                                out=xnb[p_][:, cc, h * 512 : (h + 1) * 512],
                                in0=xt[:, h, :],
                                scalar1=aa,
                                scalar2=bbn,
                                op0=OP.mult,
                                op1=OP.subtract,
                            )
                            nc.vector.tensor_scalar(
                                out=xn8[p_][:, cc, h * 512 : (h + 1) * 512],
                                in0=xt[:, h, :],
                                scalar1=aa,
                                scalar2=bbn,
                                op0=OP.mult,
                                op1=OP.subtract,
                            )
                    return _u

                for cc in range(CC):
                    units.append(load(cc))
                for cc in range(CC):
                    units.append(stats(cc))
                return units

            def qkv_group(i, oc, nci):
                """q/k channels [oc*128, +128) over n-window nci (bf16)."""
                p_ = i % 2

                def _u():
                    ps = psW.tile([128, 512], f32, tag="w", name="qk_ps")
                    for cc in range(CC):
                        nc.tensor.matmul(
                            ps,
                            cst["wqkT"][:, cc, oc * 128 : (oc + 1) * 128],
                            xnb[p_][:, cc, nci * 512 : (nci + 1) * 512],
                            start=(cc == 0),
                            stop=(cc == CC - 1),
                        )
                    nc.vector.tensor_scalar_add(
                        out=qk[p_][oc][:, nci * 512 : (nci + 1) * 512],
                        in0=ps,
                        scalar1=cst["bqk"][:, oc : oc + 1],
                    )
                return _u

            def vt_group(i, mci):
                """V^T rows [mci*128, +128) (fp8 DoubleRow; V bias folded on
                host; psum columns come out [evens | odds] and are copied
                into the [V_even|0|0|V_odd] stationary layout."""
                p_ = i % 2

                def _u():
                    ps = psW.tile([128, 512], f32, tag="w", name="vt_ps")
                    for kkx in range(KK):
                        nc.tensor.matmul(
                            ps,
                            xn8[p_][:, 2 * kkx : 2 * kkx + 2, mci * 128 : (mci + 1) * 128],
                            cst["wvT"][:, 2 * kkx : 2 * kkx + 2, :],
                            start=(kkx == 0),
                            stop=(kkx == KK - 1),
                            perf_mode=DR,
                        )
                    nc.vector.tensor_copy(
                        out=vt3[p_][:, mci, :, 0:64], in_=ps[:, 0:256]
                    )
                    nc.vector.tensor_copy(
                        out=vt3[p_][:, mci, :, 192:256], in_=ps[:, 256:512]
                    )
                return _u

            def proj_group(i, nci, oc):
                """proj channels [oc*128, +128) over n-window nci + bias + residual."""
                p_ = i % 2

                def _u():
                    ps = psW.tile([128, 512], f32, tag="w", name="pj_ps")
                    for kkx in range(KK):
                        nc.tensor.matmul(
                            ps,
                            cst["wpT"][:, 2 * kkx : 2 * kkx + 2, oc * 128 : (oc + 1) * 128],
                            o4[p_][:, 2 * kkx : 2 * kkx + 2, nci * 512 : (nci + 1) * 512],
                            start=(kkx == 0),
                            stop=(kkx == KK - 1),
                            perf_mode=DR,
                        )
                    ob = obp.tile([128, 512], f32, tag="ob", name="ob")
                    nc.vector.scalar_tensor_tensor(
                        out=ob,
                        in0=ps,
                        scalar=cst["bp"][:, oc : oc + 1],
                        in1=x4[p_][oc][:, nci, :],
                        op0=OP.add,
                        op1=OP.add,
                    )
                    nc.sync.dma_start(
                        out=d["out"][oc * 128 : (oc + 1) * 128, nci * 512 : (nci + 1) * 512],
                        in_=ob,
                    )
                return _u

            def attention_block(i, nci, q):
                """One (n-window, head-quad) block: scores+exp drumbeat with
                trailing DoubleRow attnV + denominators; the recip/psel/
                normalize tail is enqueued as filler units popped during
                the next block."""
                p_ = i % 2
                nwin = slice(nci * 512, (nci + 1) * 512)
                qkp = qk[p_]
                atts = {}
                last_den = (1, MCP - 1, 1)
                for pi in range(2):
                    att = psT.tile([128, 512], f32, tag="att", name="att")
                    atts[pi] = att
                    h0 = 4 * q + 2 * pi
                    jj = 2 * q + pi  # head-pair group in vt3 / o4 chunk
                    pts = {}
                    for mc in range(MC + 2):
                        if mc < MC:
                            sg = psA.tile([128, 1024], f32, tag="sg", name="sg")
                            for sl, h in enumerate((h0, h0 + 1)):
                                hp = (h % 2) * 64
                                nc.tensor.matmul(
                                    sg[:, sl * 512 : (sl + 1) * 512],
                                    qkp[4 + h // 2][hp : hp + 64, mc * 128 : (mc + 1) * 128],
                                    qkp[h // 2][hp : hp + 64, nwin],
                                    start=True,
                                    stop=True,
                                    tile_position=(hp, 0),
                                )
                            if mc % 2 == 0:
                                pt = ppp.tile([128, 2, 1024], f8e5, tag="pp", name="pp")
                                pts[mc // 2] = pt
                            nc.scalar.activation(
                                out=pts[mc // 2][:, mc % 2, :], in_=sg, func=AF.Exp
                            )
                        if mc >= 3 and mc % 2 == 1:
                            mcp = (mc - 3) // 2
                            pt = pts[mcp]
                            # attnV: two full-width DR matmuls with half-zero
                            # stationaries stack both heads into one bank
                            for hh in range(2):
                                nc.tensor.matmul(
                                    att,
                                    vt3[p_][:, 2 * mcp : 2 * mcp + 2, jj,
                                            hh * 128 : hh * 128 + 128],
                                    pt[:, :, hh * 512 : (hh + 1) * 512],
                                    start=(mcp == 0 and hh == 0),
                                    stop=(mcp == MCP - 1 and hh == 1),
                                    perf_mode=DR,
                                )
                            # denominators onto the pre-zeroed shared bank
                            for hh in range(2):
                                j = 2 * pi + hh
                                nc.tensor.matmul(
                                    s_ps[0 : 32 * j + 1, :],
                                    dones[:, :, 111 - 32 * j : 112],
                                    pt[:, :, hh * 512 : (hh + 1) * 512],
                                    start=False,
                                    stop=((pi, mcp, hh) == last_den),
                                    perf_mode=DR,
                                    skip_group_check=True,
                                )
                        pop_filler()

                # tail units (popped during the next block)
                def u_recip():
                    rq = rqp.tile([128, 512], bf16, tag="rq", name="rq")
                    with nc.allow_low_precision(reason="1/s feeds bf16 psel broadcast"):
                        nc.vector.reciprocal(out=rq, in_=s_ps)
                    nc.tensor.matmul(rb_ps, cst["pselA"], rq, start=True, stop=True)
                    state["rq"] = rq

                state = {}

                def u_o4(pi):
                    def _u():
                        ob4 = obp.tile([128, 512], bf16, tag="o4b", name="o4b")
                        nc.vector.tensor_copy(out=ob4, in_=atts[pi])
                        nc.vector.tensor_tensor(
                            out=o4[p_][:, 2 * q + pi, nwin],
                            in0=ob4,
                            in1=rb_ps,
                            op=OP.mult,
                        )
                    return _u

                def u_o4_pselB():
                    u_o4(0)()
                    nc.tensor.matmul(rb_ps, cst["pselB"], state["rq"], start=True, stop=True)

                def u_o4_zero():
                    u_o4(1)()
                    nc.vector.memset(s_ps, S_EPS)

                fillers.append(u_recip)
                fillers.append(u_o4_pselB)
                fillers.append(u_o4_zero)

            # ---------------- pipeline ----------------
            for i in range(unroll):
                if i == 0:
                    for u in gn_units(0):
                        u()
                    for oc in (4, 5, 6, 7, 0, 1, 2, 3):
                        for nci in range(NC):
                            qkv_group(0, oc, nci)()
                    for m in range(MC):
                        vt_group(0, m)()
                else:
                    # woven into blocks 0-1: previous iter's 2nd-window proj,
                    # this iter's 2nd-window q
                    for oc_ in range(CC):
                        fillers.append(proj_group(i - 1, 1, oc_))
                    for oc_ in (0, 1, 2, 3):
                        fillers.append(qkv_group(i, oc_, 1))

                attention_block(i, 0, 0)
                attention_block(i, 0, 1)

                # woven into blocks 2-3: this iter's 1st-window proj, next
                # iter's GN + K (both windows) + Q (1st window) + V^T
                for oc_ in range(CC):
                    fillers.append(proj_group(i, 0, oc_))
                if i + 1 < unroll:
                    fillers.extend(gn_units(i + 1))
                    for oc_ in (4, 5, 6, 7):
                        for nci_ in range(NC):
                            fillers.append(qkv_group(i + 1, oc_, nci_))
                    for oc_ in (0, 1, 2, 3):
                        fillers.append(qkv_group(i + 1, oc_, 0))
                    for m_ in range(MC):
                        fillers.append(vt_group(i + 1, m_))

                attention_block(i, 1, 0)
                attention_block(i, 1, 1)

            # tail: drain leftovers + last iteration's 2nd-window proj
            while fillers:
                fillers.popleft()()
            for oc_ in range(CC):
                proj_group(unroll - 1, 1, oc_)()
    return nc


_BUILT = None


def kernel(**inputs):
    global _BUILT
    common, xs = host_prep(**inputs)
    if _BUILT is None:
        _BUILT = build_nc(unroll=1)
    nc = _BUILT
    in_maps = [dict(common, x=xs[i]) for i in range(B)]
    res = run_bass_kernel_spmd(nc, in_maps, core_ids=list(range(B)))
    out = np.stack([res.results[i]["out"] for i in range(B)], axis=0)
    return out.reshape(B, C, 32, 32).astype(np.float32)


# revision 13
# speedup vs baseline: 1.2585x; 1.0749x over previous
"""AttentionBlock (GroupNorm + MHSA + proj + residual) on 8 Trainium2 cores.

Sharding: pure data-parallel over batch (B=8, one batch element per core).

v4 — ACT-exp drumbeat + fp8-DoubleRow attention:
  * The softmax exp stream (64 x [128,1024] activations = 8.4M elems/core)
    is the hard floor (~66us on ACT at 1 elem/cycle/lane).  Everything
    else hides under it.
  * P (exp output) is stored e5m2 (covers exp([-7.3, 7.3]) with huge
    range margin); V^T / proj weights / o4 are e4m3; q/k stay bf16
    (fp8 q/k costs ~1.2e-2 alone via exp amplification).
  * attnV runs as full-width fp8 DoubleRow matmuls with HALF-ZERO
    stationaries [V_even | 0] / [0 | V_odd], which stacks both heads of
    a pair into one PSUM bank without tile_position (walrus rejects
    DoubleRow + tile_position).  Softmax denominators use the same
    trick: [0..0 | ones] stationaries of width 32j+1 land each pair's
    sum on partition row 32j of a shared pre-zeroed bank.
  * GroupNorm stats via DVE bn_stats/bn_aggr; softmax reciprocal on DVE
    (bf16 out) feeding bf16 psel broadcast matmuls.
  * Emission is a beat-woven pipeline: scores (2-way row-packed, bf16)
    pace the ACT exps; attnV/denominator DR matmuls trail two beats
    behind; one "filler" unit per beat carries the previous block's
    recip/psel/normalize tail, proj of the previous window, and the
    next iteration's GN/qkv/V^T, with parity-double-buffered tiles.
"""

from collections import deque

import numpy as np
import ml_dtypes

import concourse.bass as bass
import concourse.tile as tile
from concourse import mybir
from concourse.bass_utils import run_bass_kernel_spmd
from concourse.vector_clock import ScopedClock, VectorClock

f32 = mybir.dt.float32
bf16 = mybir.dt.bfloat16
f8 = mybir.dt.float8e4
f8e5 = mybir.dt.float8e5
BF16 = ml_dtypes.bfloat16
F8NP = mybir.dt.np(f8)
DR = mybir.MatmulPerfMode.DoubleRow

B, C, N = 8, 512, 1024
NH, HD, G = 8, 64, 32
EPS = 1e-5
CC = C // 128   # 4 channel chunks
KK = CC // 2    # 2 DoubleRow chunk-pairs
OC_QK = 8       # q+k output chunks (1024 rows)
NC = 2          # n in two 512-windows
MC = 8          # m in eight 128-chunks
MCP = MC // 2   # 4 DoubleRow m-chunk-pairs
F8MAX = 240.0
S_EPS = 1e-30   # keeps unused s_ps rows finite through the reciprocal


def _split_multi_waits(bir_json):
    """This container's walrus build encodes at most one sync-wait command
    per TPB instruction. Engines execute in program order, so any extra
    waits can ride on NoOp instructions inserted immediately before the
    original instruction on the same engine (strictly more conservative
    ordering, semantics preserved)."""
    import orjson

    m = orjson.loads(bir_json)
    nop_id = [0]
    for fn in m.get("functions", []):
        for bb in fn.get("blocks", []):
            insts = bb.get("instructions", [])
            out = []
            for ins in insts:
                si = ins.get("sync_info") or {}
                waits = si.get("on_wait") or []
                eng = ins.get("engine", "Unassigned")
                if len(waits) > 1 and eng != "Unassigned":
                    for w in waits[:-1]:
                        nop_id[0] += 1
                        out.append(
                            {
                                "debug": ins.get("debug", 0),
                                "engine": eng,
                                "ins": [],
                                "outs": [],
                                "name": f"{ins['name']}-w{nop_id[0]}",
                                "opcode": "NoOp",
                                "sync_info": {"on_wait": [w]},
                            }
                        )
                    si = dict(si)
                    si["on_wait"] = [waits[-1]]
                    ins = dict(ins)
                    ins["sync_info"] = si
                out.append(ins)
            bb["instructions"] = out
    return orjson.dumps(m)


def _patch_tile():
    """This container's walrus accepts few sem-waits per instruction; split
    TileContext's kernel-tail drain into one drain per pending proc and
    hoist any remaining multi-waits onto NoOps at compile time."""
    if getattr(tile.TileContext, "_drain_split_patched", False):
        return

    from concourse import bass2jax, bass_utils

    orig_compile = bass_utils.compile_bir_kernel

    def compile_with_split(bir_json, tmpdir, neff_name="file.neff"):
        return orig_compile(_split_multi_waits(bir_json), tmpdir, neff_name=neff_name)

    bass_utils.compile_bir_kernel = compile_with_split
    bass2jax.compile_bir_kernel = compile_with_split

    def _drain_and_barrier_split(self, tick_clock, wait_clock):
        gc = tick_clock.global_clock
        ticks = list(gc)
        for p, t in enumerate(ticks):
            if t <= 0:
                continue
            vec = [0] * len(ticks)
            vec[p] = t
            drain_inst = self.nc.sync.drain()
            wait_clock.add_sem_waits(
                drain_inst.ins, ScopedClock({None: VectorClock(vec)})
            )
        self.nc.all_engine_barrier()
        assert self.sems is not None
        popped = self.nc._tile_sem_poison_stack.pop()
        assert popped is self._sem_poison
        self.nc.clear_and_free_semaphores(list(self.sems.allocated().values()))
        self.nc.all_engine_barrier()

    tile.TileContext._drain_and_barrier = _drain_and_barrier_split
    tile.TileContext._drain_split_patched = True


def _to_f8(a):
    return np.clip(np.asarray(a, np.float32), -F8MAX, F8MAX).astype(F8NP)


# even heads first, then odd heads: V^T psum columns come out pre-grouped
# for the [V_even | 0 | 0 | V_odd] stationary layout
_VPERM = [0, 2, 4, 6, 1, 3, 5, 7]


def host_prep(x, norm_w, norm_b, qkv_w, qkv_b, proj_w, proj_b):
    """Host-side layout/dtype prep.  Folds the 1/sqrt(hd) attention scale
    into W_q/b_q (exact: 0.125 is a power of 2) and the V bias through the
    softmax (rows sum to 1) into the projection bias:
    bp' = proj_b + proj_w @ b_v."""
    x = np.ascontiguousarray(np.asarray(x, np.float32)).reshape(B, C, N)
    qkv_w = np.asarray(qkv_w, np.float32)
    qkv_b = np.asarray(qkv_b, np.float32)
    proj_w = np.asarray(proj_w, np.float32)
    proj_b = np.asarray(proj_b, np.float32)
    scale = float(HD) ** -0.5

    wqk = qkv_w[: 2 * C].copy()
    wqk[:C] *= scale
    bqk = qkv_b[: 2 * C].copy()
    bqk[:C] *= scale
    bp_eff = proj_b + proj_w @ qkv_b[2 * C :]

    # permute V output channels: even heads' 64-blocks first, then odd
    wv = qkv_w[2 * C :].reshape(NH, HD, C)[_VPERM].reshape(C, C)

    common = {
        "wqkT": np.ascontiguousarray(wqk.T).astype(BF16),   # [512, 1024]
        "wvT": _to_f8(wv.T),                                # [512, 512]
        "wpT": _to_f8(proj_w.T),                            # [512, 512]
        "bqk": np.ascontiguousarray(bqk.reshape(OC_QK, 128).T).astype(np.float32),
        "bp": np.ascontiguousarray(bp_eff.reshape(CC, 128).T).astype(np.float32),
        "nw": np.ascontiguousarray(
            np.asarray(norm_w, np.float32).reshape(CC, 128).T
        ).astype(np.float32),
        "nb": np.ascontiguousarray(
            np.asarray(norm_b, np.float32).reshape(CC, 128).T
        ).astype(np.float32),
        "gmat": _gmat(),
        "pselA": _psel(0, 32).astype(BF16),
        "pselB": _psel(64, 96).astype(BF16),
    }
    return common, [x[i] for i in range(B)]


def _gmat():
    # mixes per-channel [mean, E[x^2]] into per-group averages (16 ch/group)
    g = np.zeros((128, 128), np.float32)
    for i in range(128):
        gi = i // 16
        g[i, gi * 16 : (gi + 1) * 16] = 1.0 / 16.0
    return g


def _psel(r0, r1):
    p = np.zeros((128, 128), np.float32)
    p[r0, 0:64] = 1.0
    p[r1, 64:128] = 1.0
    return p


def build_nc(unroll=1):
    _patch_tile()
    nc = bass.Bass()
    d = {}
    d["x"] = nc.declare_dram_parameter("x", [C, N], f32, isOutput=False)
    d["wqkT"] = nc.declare_dram_parameter("wqkT", [C, 2 * C], bf16, isOutput=False)
    d["wvT"] = nc.declare_dram_parameter("wvT", [C, C], f8, isOutput=False)
    d["wpT"] = nc.declare_dram_parameter("wpT", [C, C], f8, isOutput=False)
    d["bqk"] = nc.declare_dram_parameter("bqk", [128, OC_QK], f32, isOutput=False)
    d["bp"] = nc.declare_dram_parameter("bp", [128, CC], f32, isOutput=False)
    d["nw"] = nc.declare_dram_parameter("nw", [128, CC], f32, isOutput=False)
    d["nb"] = nc.declare_dram_parameter("nb", [128, CC], f32, isOutput=False)
    d["gmat"] = nc.declare_dram_parameter("gmat", [128, 128], f32, isOutput=False)
    d["pselA"] = nc.declare_dram_parameter("pselA", [128, 128], bf16, isOutput=False)
    d["pselB"] = nc.declare_dram_parameter("pselB", [128, 128], bf16, isOutput=False)
    d["out"] = nc.declare_dram_parameter("out", [C, N], f32, isOutput=True)

    OP = mybir.AluOpType
    AF = mybir.ActivationFunctionType

    with tile.TileContext(nc) as tc:
        with (
            tc.tile_pool(name="sing", bufs=1) as sing,
            tc.tile_pool(name="gn", bufs=4) as gnp,
            tc.tile_pool(name="pp", bufs=12) as ppp,
            tc.tile_pool(name="rqp", bufs=2) as rqp,
            tc.tile_pool(name="obp", bufs=4) as obp,
            tc.tile_pool(name="psA", bufs=2, space="PSUM") as psA,
            tc.tile_pool(name="psT", bufs=2, space="PSUM") as psT,
            tc.tile_pool(name="psW", bufs=1, space="PSUM") as psW,
            tc.tile_pool(name="psD", bufs=1, space="PSUM") as psD,
        ):
            # ---- constants ----
            cst = {}
            for name, shape, dt in (
                ("wqkT", [128, CC, 2 * C], bf16),
                ("wvT", [128, CC, C], f8),
                ("wpT", [128, CC, C], f8),
                ("bqk", [128, OC_QK], f32),
                ("bp", [128, CC], f32),
                ("nw", [128, CC], f32),
                ("nb", [128, CC], f32),
                ("gmat", [128, 128], f32),
                ("pselA", [128, 128], bf16),
                ("pselB", [128, 128], bf16),
            ):
                t = sing.tile(shape, dt, tag=name, name=name)
                src = d[name]
                if len(shape) == 3:
                    nc.sync.dma_start(
                        out=t, in_=src.rearrange("(cc p) o -> p cc o", p=128)
                    )
                else:
                    nc.sync.dma_start(out=t, in_=src[:])
                cst[name] = t
            eps_t = sing.tile([128, 1], f32, tag="eps", name="eps")
            nc.vector.memset(eps_t, EPS)
            # M=1 stationary for the 4-way col-packed denominator matmuls
            ones5 = sing.tile([128, 1], f8e5, tag="ones5", name="ones5")
            nc.vector.memset(ones5, 1.0)

            s_ps = psD.tile([128, 512], f32, tag="s", name="s_ps")
            rb_ps = s_ps  # rb broadcasts reuse the sums bank after recip
            nc.vector.memset(s_ps, S_EPS)

            # ---- per-parity state tiles ----
            x4, xnb, xn8, qk, vt3, o4 = {}, {}, {}, {}, {}, {}
            for p_ in range(2):
                x4[p_] = [
                    sing.tile([128, NC, 512], f32, tag=f"x{cc}_{p_}", name=f"x{cc}_{p_}")
                    for cc in range(CC)
                ]
                xnb[p_] = sing.tile([128, CC, N], bf16, tag=f"xnb_{p_}", name=f"xnb_{p_}")
                xn8[p_] = sing.tile([128, CC, N], f8, tag=f"xn8_{p_}", name=f"xn8_{p_}")
                qk[p_] = [
                    sing.tile([128, N], bf16, tag=f"qk{oc}_{p_}", name=f"qk{oc}_{p_}")
                    for oc in range(OC_QK)
                ]
                # [V_even | 0 | 0 | V_odd] per head-pair group of 256 cols
                vt3[p_] = sing.tile(
                    [128, MC, 4, 256], f8, tag=f"vt3_{p_}", name=f"vt3_{p_}"
                )
                nc.vector.memset(vt3[p_], 0.0)
                o4[p_] = sing.tile([128, CC, N], f8, tag=f"o4_{p_}", name=f"o4_{p_}")

            fillers = deque()

            def pop_filler():
                if fillers:
                    fillers.popleft()()

            # ---------------- stage emitters ----------------
            def gn_units(i):
                """GroupNorm of iteration i as a list of filler units."""
                p_ = i % 2
                units = []

                def load(cc):
                    def _u():
                        nc.gpsimd.dma_start(
                            out=x4[p_][cc],
                            in_=d["x"][cc * 128 : (cc + 1) * 128, :].rearrange(
                                "p (s n) -> p s n", s=NC
                            ),
                        )
                    return _u

                def stats(cc):
                    def _u():
                        xt = x4[p_][cc]
                        st6 = gnp.tile([128, NC * 6], f32, tag="st6", name="st6")
                        for h in range(NC):
                            nc.vector.bn_stats(st6[:, h * 6 : (h + 1) * 6], xt[:, h, :])
                        ms = gnp.tile([128, 2], f32, tag="ms", name="ms")
                        nc.vector.bn_aggr(ms, st6)
                        # ms = [mean_c, var_c] -> ex2 = [mean_c, E[x^2]_c]
                        ex2 = gnp.tile([128, 2], f32, tag="ex2", name="ex2")
                        nc.vector.tensor_copy(out=ex2[:, 0:1], in_=ms[:, 0:1])
                        nc.vector.scalar_tensor_tensor(
                            out=ex2[:, 1:2],
                            in0=ms[:, 0:1],
                            scalar=ms[:, 0:1],
                            in1=ms[:, 1:2],
                            op0=OP.mult,
                            op1=OP.add,
                        )
                        gst_ps = psW.tile([128, 2], f32, tag="w", name="gst_ps")
                        nc.tensor.matmul(gst_ps, cst["gmat"], ex2, start=True, stop=True)
                        gst = gnp.tile([128, 2], f32, tag="gst", name="gst")
                        nc.vector.tensor_copy(out=gst, in_=gst_ps)
                        negvar = gnp.tile([128, 1], f32, tag="negvar", name="negvar")
                        nc.vector.scalar_tensor_tensor(
                            out=negvar,
                            in0=gst[:, 0:1],
                            scalar=gst[:, 0:1],
                            in1=gst[:, 1:2],
                            op0=OP.mult,
                            op1=OP.subtract,
                        )
                        rstd = gnp.tile([128, 1], f32, tag="rstd", name="rstd")
                        nc.scalar.activation(
                            out=rstd, in_=negvar, func=AF.Sqrt, bias=eps_t, scale=-1.0
                        )
                        nc.vector.reciprocal(out=rstd, in_=rstd)
                        aa = gnp.tile([128, 1], f32, tag="aa", name="aa")
                        nc.vector.tensor_mul(out=aa, in0=rstd, in1=cst["nw"][:, cc : cc + 1])
                        bbn = gnp.tile([128, 1], f32, tag="bbn", name="bbn")
                        nc.vector.scalar_tensor_tensor(
                            out=bbn,
                            in0=gst[:, 0:1],
                            scalar=aa,
                            in1=cst["nb"][:, cc : cc + 1],
                            op0=OP.mult,
                            op1=OP.subtract,
                        )
                        for h in range(NC):
                            nc.vector.tensor_scalar(
                                out=xnb[p_][:, cc, h * 512 : (h + 1) * 512],
                                in0=xt[:, h, :],
                                scalar1=aa,
                                scalar2=bbn,
                                op0=OP.mult,
                                op1=OP.subtract,
                            )
                            nc.vector.tensor_scalar(
                                out=xn8[p_][:, cc, h * 512 : (h + 1) * 512],
                                in0=xt[:, h, :],
                                scalar1=aa,
                                scalar2=bbn,
                                op0=OP.mult,
                                op1=OP.subtract,
                            )
                    return _u

                for cc in range(CC):
                    units.append(load(cc))
                for cc in range(CC):
                    units.append(stats(cc))
                return units

            def qkv_group(i, oc, nci):
                """q/k channels [oc*128, +128) over n-window nci (bf16)."""
                p_ = i % 2

                def _u():
                    ps = psW.tile([128, 512], f32, tag="w", name="qk_ps")
                    for cc in range(CC):
                        nc.tensor.matmul(
                            ps,
                            cst["wqkT"][:, cc, oc * 128 : (oc + 1) * 128],
                            xnb[p_][:, cc, nci * 512 : (nci + 1) * 512],
                            start=(cc == 0),
                            stop=(cc == CC - 1),
                        )
                    nc.vector.tensor_scalar_add(
                        out=qk[p_][oc][:, nci * 512 : (nci + 1) * 512],
                        in0=ps,
                        scalar1=cst["bqk"][:, oc : oc + 1],
                    )
                return _u

            def vt_group(i, mci):
                """V^T rows [mci*128, +128) (fp8 DoubleRow; V bias folded on
                host; psum columns come out [evens | odds] and are copied
                into the [V_even|0|0|V_odd] stationary layout."""
                p_ = i % 2

                def _u():
                    ps = psW.tile([128, 512], f32, tag="w", name="vt_ps")
                    for kkx in range(KK):
                        nc.tensor.matmul(
                            ps,
                            xn8[p_][:, 2 * kkx : 2 * kkx + 2, mci * 128 : (mci + 1) * 128],
                            cst["wvT"][:, 2 * kkx : 2 * kkx + 2, :],
                            start=(kkx == 0),
                            stop=(kkx == KK - 1),
                            perf_mode=DR,
                        )
                    nc.vector.tensor_copy(
                        out=vt3[p_][:, mci, :, 0:64], in_=ps[:, 0:256]
                    )
                    nc.vector.tensor_copy(
                        out=vt3[p_][:, mci, :, 192:256], in_=ps[:, 256:512]
                    )
                return _u

            def proj_group(i, nci, oc):
                """proj channels [oc*128, +128) over n-window nci + bias + residual."""
                p_ = i % 2

                def _u():
                    ps = psW.tile([128, 512], f32, tag="w", name="pj_ps")
                    for kkx in range(KK):
                        nc.tensor.matmul(
                            ps,
                            cst["wpT"][:, 2 * kkx : 2 * kkx + 2, oc * 128 : (oc + 1) * 128],
                            o4[p_][:, 2 * kkx : 2 * kkx + 2, nci * 512 : (nci + 1) * 512],
                            start=(kkx == 0),
                            stop=(kkx == KK - 1),
                            perf_mode=DR,
                        )
                    ob = obp.tile([128, 512], f32, tag="ob", name="ob")
                    nc.vector.scalar_tensor_tensor(
                        out=ob,
                        in0=ps,
                        scalar=cst["bp"][:, oc : oc + 1],
                        in1=x4[p_][oc][:, nci, :],
                        op0=OP.add,
                        op1=OP.add,
                    )
                    nc.sync.dma_start(
                        out=d["out"][oc * 128 : (oc + 1) * 128, nci * 512 : (nci + 1) * 512],
                        in_=ob,
                    )
                return _u

            def attention_block(i, nci, q):
                """One (n-window, head-quad) block: scores+exp drumbeat with
                trailing DoubleRow attnV + denominators; the recip/psel/
                normalize tail is enqueued as filler units popped during
                the next block."""
                p_ = i % 2
                nwin = slice(nci * 512, (nci + 1) * 512)
                qkp = qk[p_]
                atts = {}
                ptsall = {0: {}, 1: {}}
                for pi in range(2):
                    att = psT.tile([128, 512], f32, tag="att", name="att")
                    atts[pi] = att
                    h0 = 4 * q + 2 * pi
                    jj = 2 * q + pi  # head-pair group in vt3 / o4 chunk
                    pts = ptsall[pi]
                    for mc in range(MC + 2):
                        if mc < MC:
                            sg = psA.tile([128, 1024], f32, tag="sg", name="sg")
                            for sl, h in enumerate((h0, h0 + 1)):
                                hp = (h % 2) * 64
                                nc.tensor.matmul(
                                    sg[:, sl * 512 : (sl + 1) * 512],
                                    qkp[4 + h // 2][hp : hp + 64, mc * 128 : (mc + 1) * 128],
                                    qkp[h // 2][hp : hp + 64, nwin],
                                    start=True,
                                    stop=True,
                                    tile_position=(hp, 0),
                                )
                            if mc % 2 == 0:
                                pt = ppp.tile([128, 2, 1024], f8e5, tag="pp", name="pp")
                                pts[mc // 2] = pt
                            nc.scalar.activation(
                                out=pts[mc // 2][:, mc % 2, :], in_=sg, func=AF.Exp
                            )
                        if mc >= 3 and mc % 2 == 1:
                            mcp = (mc - 3) // 2
                            pt = pts[mcp]
                            # attnV: two full-width DR matmuls with half-zero
                            # stationaries stack both heads into one bank
                            for hh in range(2):
                                nc.tensor.matmul(
                                    att,
                                    vt3[p_][:, 2 * mcp : 2 * mcp + 2, jj,
                                            hh * 128 : hh * 128 + 128],
                                    pt[:, :, hh * 512 : (hh + 1) * 512],
                                    start=(mcp == 0 and hh == 0),
                                    stop=(mcp == MCP - 1 and hh == 1),
                                    perf_mode=DR,
                                )
                        if pi == 1 and mc >= 2:
                            # denominators: 4-way col-tiled M=1 matmuls over
                            # both pairs' pt slabs (normal mode, e5m2)
                            mcd = mc - 2
                            for pj in range(2):
                                ptj = ptsall[pj][mcd // 2]
                                for hh in range(2):
                                    j = 2 * pj + hh
                                    nc.tensor.matmul(
                                        s_ps[32 * j : 32 * j + 1, :],
                                        ones5,
                                        ptj[:, mcd % 2, hh * 512 : (hh + 1) * 512],
                                        start=(mcd == 0),
                                        stop=(mcd == MC - 1),
                                        tile_position=(0, 32 * j),
                                        skip_group_check=True,
                                    )
                        pop_filler()

                # tail units (popped during the next block)
                def u_recip():
                    rq = rqp.tile([128, 512], bf16, tag="rq", name="rq")
                    with nc.allow_low_precision(reason="1/s feeds bf16 psel broadcast"):
                        nc.vector.reciprocal(out=rq, in_=s_ps)
                    nc.tensor.matmul(rb_ps, cst["pselA"], rq, start=True, stop=True)
                    state["rq"] = rq

                state = {}

                def u_o4(pi):
                    def _u():
                        ob4 = obp.tile([128, 512], bf16, tag="o4b", name="o4b")
                        nc.vector.tensor_copy(out=ob4, in_=atts[pi])
                        nc.vector.tensor_tensor(
                            out=o4[p_][:, 2 * q + pi, nwin],
                            in0=ob4,
                            in1=rb_ps,
                            op=OP.mult,
                        )
                    return _u

                def u_o4_pselB():
                    u_o4(0)()
                    nc.tensor.matmul(rb_ps, cst["pselB"], state["rq"], start=True, stop=True)

                fillers.append(u_recip)
                fillers.append(u_o4_pselB)
                fillers.append(u_o4(1))

            # ---------------- pipeline ----------------
            for i in range(unroll):
                if i == 0:
                    for u in gn_units(0):
                        u()
                    for oc in (4, 5, 6, 7, 0, 1, 2, 3):
                        for nci in range(NC):
                            qkv_group(0, oc, nci)()
                    for m in range(MC):
                        vt_group(0, m)()
                else:
                    # woven into blocks 0-1: previous iter's 2nd-window proj,
                    # this iter's 2nd-window q
                    for oc_ in range(CC):
                        fillers.append(proj_group(i - 1, 1, oc_))
                    for oc_ in (0, 1, 2, 3):
                        fillers.append(qkv_group(i, oc_, 1))

                attention_block(i, 0, 0)
                attention_block(i, 0, 1)

                # woven into blocks 2-3: this iter's 1st-window proj, next
                # iter's GN + K (both windows) + Q (1st window) + V^T
                for oc_ in range(CC):
                    fillers.append(proj_group(i, 0, oc_))
                if i + 1 < unroll:
                    fillers.extend(gn_units(i + 1))
                    for oc_ in (4, 5, 6, 7):
                        for nci_ in range(NC):
                            fillers.append(qkv_group(i + 1, oc_, nci_))
                    for oc_ in (0, 1, 2, 3):
                        fillers.append(qkv_group(i + 1, oc_, 0))
                    for m_ in range(MC):
                        fillers.append(vt_group(i + 1, m_))

                attention_block(i, 1, 0)
                attention_block(i, 1, 1)

            # tail: drain leftovers + last iteration's 2nd-window proj
            while fillers:
                fillers.popleft()()
            for oc_ in range(CC):
                proj_group(unroll - 1, 1, oc_)()
    return nc


_BUILT = None


def kernel(**inputs):
    global _BUILT
    common, xs = host_prep(**inputs)
    if _BUILT is None:
        _BUILT = build_nc(unroll=1)
    nc = _BUILT
    in_maps = [dict(common, x=xs[i]) for i in range(B)]
    res = run_bass_kernel_spmd(nc, in_maps, core_ids=list(range(B)))
    out = np.stack([res.results[i]["out"] for i in range(B)], axis=0)
    return out.reshape(B, C, 32, 32).astype(np.float32)
